# revision 29
# baseline (speedup 1.0000x reference)
"""Trainium2 Bass kernel for nn_AUCDomainAdapation (AUC domain-adaptation loss).

Contract: kernel(**inputs) takes the FULL unsharded inputs of reference.setup_inputs()
and returns the same structure as reference.reference(**inputs): a tuple
(0.25*empirical, transfer) of float32 scalars.

Math (validated vs reference):
  For sample i with label c_i, the masked pairwise sum per term reduces to
    sum_j M_ij * L(Q_ij),  Q_ij = q0_i + s * B'_ij
  where B'_ij = (s*Y_shard @ pTm)_ij and pTm[c, j] = p[j, c] * [label_j != c]
  (class-major softmax probs with same-label columns zeroed).  Masked entries
  hit B' = 0 exactly, so the unmasked row-sum minus n_{c_i} * L(q0_i) is the
  masked sum.  L(Q) = softplus(-Q) + softplus(Q + 2*eps)
              = ln((1 + e^{2 eps}) + e^{Q+2 eps} + e^{-Q}).

Sharding: row-shard the 1024 samples over 8 cores (128 rows each).  Inputs are
rotated per-core along the sample axis so a single SPMD program (slice [0:128])
serves all cores; per-core scalar partials are summed on the host.
"""

import numpy as np
from contextlib import ExitStack

import concourse.bass as bass
import concourse.bacc as bacc
import concourse.tile as tile
from concourse import mybir
from concourse.bass_utils import run_bass_kernel_spmd
from concourse._compat import with_exitstack

F32 = mybir.dt.float32
BF16 = mybir.dt.bfloat16
AF = mybir.ActivationFunctionType
ALU = mybir.AluOpType

N = 1024          # samples
C = 64            # classes
SH = 128          # rows per core
NCORES = 8
EPS = 0.05
C0 = float(1.0 + np.exp(2 * EPS))   # 1 + e^{2eps}
SE = float(np.exp(EPS))             # e^{eps}
CHUNK = 512                         # matmul moving free-dim max

INPUT_SPECS = {
    "yT_sx":   ([128, N], BF16),   # [y_s^T ; y_s_adv^T] class-major stack
    "yT_tx":   ([128, N], BF16),   # [y_t^T ; y_t_adv^T]
    "yta_nat": ([128, 512], F32),  # y_t_adv natural, j=(a,p): [p, a*64+c]
    "y4_nat":  ([128, 256], F32),  # shard rows: [y_s | y_sa | y_t | y_ta]
    # packed constants:
    # smalls: ident[0:128] | lab_sh[128] | iota_mod[129] | lhA-bits[130:132] | lhB-bits[132:134]
    "smalls":  ([128, 134], F32),
    # rows1: lab_row[0:1024] | ones1[1024:1152] (bf16: labels <= 63 exact)
    "rows1":   ([1, 1152], BF16),
    # rows2: bf16 bits of [lbS | lbT] ([2, 256] bf16)
    "rows2":   ([2, 128], F32),
}


def _patch_act_tables():
    """Force exp+ln to resolve to the single natural_log_exp_and_others set,
    avoiding table reloads between Exp and Ln activations."""
    if getattr(bacc, "_act_tables_patched", False):
        return
    orig = bacc.get_activation_tables

    def patched(arch):
        tabs = dict(orig(arch))
        out = {}
        for name, funcs in tabs.items():
            if name != "natural_log_exp_and_others":
                funcs = {f for f in funcs if f not in (AF.Exp, AF.Ln)}
            out[name] = funcs
        return out

    bacc.get_activation_tables = patched
    bacc._act_tables_patched = True


@with_exitstack
def _body(ctx: ExitStack, tc: "tile.TileContext", ins: dict, out2: "bass.AP", beta_t: float):
    nc = tc.nc
    sb = ctx.enter_context(tc.tile_pool(name="sb", bufs=1))
    sw = ctx.enter_context(tc.tile_pool(name="sw", bufs=3))   # loss scratch
    psT = ctx.enter_context(tc.tile_pool(name="psT", bufs=2, space="PSUM"))
    psP = ctx.enter_context(tc.tile_pool(name="psP", bufs=2, space="PSUM"))

    # ---- input loads (packed; spread over both HWDGE engines) --------------
    t_ysx = sb.tile([128, N], BF16)
    nc.sync.dma_start(t_ysx[:], ins["yT_sx"][:])
    t_ytx = sb.tile([128, N], BF16)
    nc.sync.dma_start(t_ytx[:], ins["yT_tx"][:])
    t_rows1 = sb.tile([1, 1152], BF16)
    nc.gpsimd.dma_start(t_rows1[:], ins["rows1"][:])
    t_smalls = sb.tile([128, 134], F32)
    nc.gpsimd.dma_start(t_smalls[:], ins["smalls"][:])
    t_y4 = sb.tile([128, 256], F32)
    nc.gpsimd.dma_start(t_y4[:], ins["y4_nat"][:])
    t_ytan = sb.tile([128, 512], F32)
    nc.gpsimd.dma_start(t_ytan[:], ins["yta_nat"][:])
    t_rows2 = sb.tile([2, 128], F32)
    nc.gpsimd.dma_start(t_rows2[:], ins["rows2"][:])
    t_ident = t_smalls[:, 0:128]
    t_labsh = t_smalls[:, 128:129]
    t_iotam = t_smalls[:, 129:130]
    t_lhA = t_smalls[:, 130:132].bitcast(BF16)
    t_lhB = t_smalls[:, 132:134].bitcast(BF16)
    t_lab1 = t_rows1[0:1, 0:N]
    t_ones1 = t_rows1[0:1, N:N + 128]
    _rows2b = t_rows2[:].bitcast(BF16)
    t_lbS = _rows2b[:, 0:128]
    t_lbT = _rows2b[:, 128:256]

    # ---- small constants --------------------------------------------------
    t_epsb = sb.tile([128, 1], F32)
    nc.gpsimd.memset(t_epsb[:], EPS)
    t_c0b = sb.tile([128, 1], F32)
    nc.gpsimd.memset(t_c0b[:], C0)
    t_ones = sb.tile([128, 1], F32)
    nc.gpsimd.memset(t_ones[:], 1.0)

    # ---- source exp + colsum + reciprocal chain (critical path) -------------
    t_Exs = sb.tile([128, N], BF16)
    nc.scalar.activation(t_Exs[:], t_ysx[:], AF.Exp)
    t_ey4 = sb.tile([128, 256], F32)
    nc.scalar.activation(t_ey4[:], t_y4[:], AF.Exp)
    p_cs_s = psP.tile([2, N], F32, tag="prep")
    for ch in range(2):
        sl = slice(ch * CHUNK, (ch + 1) * CHUNK)
        nc.tensor.matmul(p_cs_s[:, sl], t_lhA[:, 0:2], t_Exs[:, sl], start=True, stop=True)
    t_lnS_s = sb.tile([2, N], F32)
    t_R2b_s = sb.tile([2, N], BF16)
    for ch in range(2):
        sl = slice(ch * CHUNK, (ch + 1) * CHUNK)
        nc.scalar.activation(t_lnS_s[:, sl], p_cs_s[:, sl], AF.Ln)
        nc.scalar.activation(t_R2b_s[:, sl], t_lnS_s[:, sl], AF.Exp, scale=-1.0)

    # ---- per-row chain: a_i, q0, exp biases (gates the first loss exp) ------
    t_m8 = sb.tile([128, 8], F32)
    ytan_v = t_ytan[:].rearrange("p (a c) -> p a c", c=64)
    nc.vector.tensor_reduce(t_m8[:], ytan_v, axis=mybir.AxisListType.X, op=ALU.max)
    t_oh = sb.tile([128, 512], F32)
    nc.vector.tensor_tensor(t_oh[:].rearrange("p (a c) -> p a c", c=64), ytan_v,
                            t_m8[:, :, None].broadcast_to((128, 8, 64)),
                            op=ALU.is_equal)
    t_den4 = sb.tile([128, 4], F32)
    nc.vector.tensor_reduce(t_den4[:], t_ey4[:].rearrange("p (a c) -> p a c", c=64),
                            axis=mybir.AxisListType.X, op=ALU.add)
    t_rec4 = sb.tile([128, 4], F32)
    nc.vector.reciprocal(t_rec4[:], t_den4[:])
    t_iotaf = sb.tile([128, 64], F32)
    nc.gpsimd.iota(t_iotaf[:], pattern=[[1, 64]], base=0, channel_multiplier=0,
                   allow_small_or_imprecise_dtypes=True)
    t_Ysn = sb.tile([128, 64], F32)
    nc.vector.tensor_scalar(t_Ysn[:], t_iotaf[:], t_labsh, None, op0=ALU.is_equal)
    t_Y4n = sb.tile([128, 256], F32)
    nc.gpsimd.tensor_copy(t_Y4n[:, 0:64], t_Ysn[:])
    nc.gpsimd.tensor_copy(t_Y4n[:, 64:128], t_Ysn[:])
    nc.gpsimd.tensor_copy(t_Y4n[:, 128:192], t_oh[:, 0:64])
    nc.gpsimd.tensor_copy(t_Y4n[:, 192:256], t_oh[:, 0:64])
    t_scr = sb.tile([128, 256], F32)
    nc.gpsimd.tensor_tensor(t_scr[:], t_Y4n[:], t_y4[:], op=ALU.mult)
    t_sel4 = sb.tile([128, 4], F32)
    nc.vector.tensor_reduce(t_sel4[:], t_scr[:].rearrange("p (a c) -> p a c", c=64),
                            axis=mybir.AxisListType.X, op=ALU.add)
    t_es4 = sb.tile([128, 4], F32)
    nc.scalar.activation(t_es4[:], t_sel4[:], AF.Exp)
    t_a4 = sb.tile([128, 4], F32)
    nc.vector.tensor_tensor(t_a4[:], t_es4[:], t_rec4[:], op=ALU.mult)
    t_q0 = sb.tile([128, 3], F32)
    nc.vector.tensor_scalar(t_q0[:, 0:1], t_a4[:, 0:1], -4.0, 4.0 - EPS,
                            op0=ALU.mult, op1=ALU.add)
    t_u = sb.tile([128, 1], F32, tag="u1")
    nc.vector.tensor_tensor(t_u[:], t_a4[:, 1:2], t_a4[:, 0:1], op=ALU.subtract)
    nc.vector.tensor_scalar(t_q0[:, 1:2], t_u[:], 2.0, -EPS, op0=ALU.mult, op1=ALU.add)
    t_u2 = sb.tile([128, 1], F32, tag="u2")
    nc.vector.tensor_tensor(t_u2[:], t_a4[:, 3:4], t_a4[:, 2:3], op=ALU.subtract)
    nc.vector.tensor_scalar(t_q0[:, 2:3], t_u2[:], 2.0, -EPS, op0=ALU.mult, op1=ALU.add)
    t_b1e = sb.tile([128, 1], F32)
    nc.vector.tensor_scalar(t_b1e[:], t_q0[:, 0:1], EPS, None, op0=ALU.add)
    t_b2 = sb.tile([128, 3], F32)
    nc.vector.tensor_scalar(t_b2[:], t_q0[:], 2.0 * EPS, None, op0=ALU.add)
    t_bn = sb.tile([128, 3], F32)
    nc.vector.tensor_scalar(t_bn[:], t_q0[:], -1.0, None, op0=ALU.mult)

    # ---- target exp + colsum + reciprocal chain -----------------------------
    t_Ext = sb.tile([128, N], BF16)
    nc.scalar.activation(t_Ext[:], t_ytx[:], AF.Exp)
    p_cs_t = psP.tile([2, N], F32, tag="prep")
    for ch in range(2):
        sl = slice(ch * CHUNK, (ch + 1) * CHUNK)
        nc.tensor.matmul(p_cs_t[:, sl], t_lhB[:, 2:4], t_Ext[:, sl], start=True, stop=True)
    t_lnS_t = sb.tile([2, N], F32)
    nc.scalar.activation(t_lnS_t[:], p_cs_t[:], AF.Ln)
    t_R2b_t = sb.tile([2, N], BF16)
    nc.scalar.activation(t_R2b_t[:], t_lnS_t[:], AF.Exp, scale=-1.0)

    # ---- source one-hot stack (label broadcast via PE, borrowed T slot) -----
    p_lab128 = psT.tile([128, N], F32, tag="T")
    for ch in range(2):
        sl = slice(ch * CHUNK, (ch + 1) * CHUNK)
        nc.tensor.matmul(p_lab128[:, sl], t_ones1, t_lab1[:, sl], start=True, stop=True)
    t_YTsx = sb.tile([128, N], F32)
    t_ns = sb.tile([128, 1], F32)
    nc.vector.tensor_scalar(t_YTsx[:], p_lab128[:], t_iotam, None,
                            op0=ALU.is_equal, op1=ALU.add, accum_out=t_ns[:])
    t_lhemp = sb.tile([64, 128], BF16)
    nc.vector.tensor_scalar(t_lhemp[:], t_YTsx[0:64, 0:SH], 4.0, None, op0=ALU.mult)
    t_lh2s = sb.tile([64, 128], BF16)
    nc.vector.tensor_scalar(t_lh2s[:], t_YTsx[0:64, 0:SH], 2.0, None, op0=ALU.mult)
    t_lhm2s = sb.tile([64, 128], BF16)
    nc.vector.tensor_scalar(t_lhm2s[:], t_YTsx[0:64, 0:SH], -2.0, None, op0=ALU.mult)

    # ---- source masked probs -------------------------------------------------
    p_rbs = psP.tile([128, N], F32, tag="prep")
    for ch in range(2):
        sl = slice(ch * CHUNK, (ch + 1) * CHUNK)
        nc.tensor.matmul(p_rbs[:, sl], t_lbS, t_R2b_s[:, sl], start=True, stop=True)
    t_RMs = sb.tile([128, N], BF16)
    t_PTs_lo = sb.tile([64, N], BF16)
    t_PTs_hi = sb.tile([64, N], BF16)
    for ch in range(2):
        sl = slice(ch * CHUNK, (ch + 1) * CHUNK)
        nc.vector.scalar_tensor_tensor(t_RMs[:, sl], t_YTsx[:, sl], 0.0, p_rbs[:, sl],
                                       op0=ALU.is_equal, op1=ALU.mult)
        nc.vector.tensor_tensor(t_PTs_lo[:, sl], t_Exs[0:64, sl], t_RMs[0:64, sl], op=ALU.mult)
        nc.vector.tensor_tensor(t_PTs_hi[:, sl], t_Exs[64:128, sl], t_RMs[64:128, sl], op=ALU.mult)

    # ---- emp + src terms (start while target prep continues) ----------------
    t_as = sb.tile([128, 6], F32)

    def term_chunked(base_col, bias2_ap, biasn_ap, mm_emit):
        t_eu = sw.tile([128, N], F32, tag="eu")
        t_ev = sw.tile([128, N], F32, tag="ev")
        t_z = sw.tile([128, N], F32, tag="z")
        t_l = sw.tile([128, N], F32, tag="l")
        for ch in range(2):
            sl = slice(ch * CHUNK, (ch + 1) * CHUNK)
            mm_emit(ch, sl)
            nc.scalar.activation(t_eu[:, sl], mm_emit.out[:, sl], AF.Exp,
                                 bias=bias2_ap, scale=1.0)
            nc.scalar.activation(t_ev[:, sl], mm_emit.out[:, sl], AF.Exp,
                                 bias=biasn_ap, scale=-1.0)
            nc.vector.tensor_tensor(t_z[:, sl], t_eu[:, sl], t_ev[:, sl], op=ALU.add)
            nc.scalar.activation(t_l[:, sl], t_z[:, sl], AF.Ln, bias=t_c0b[:], scale=1.0,
                                 accum_out=t_as[:, base_col + ch:base_col + ch + 1])

    p_Te = psT.tile([128, N], F32, tag="T")

    def mm_emp(ch, sl):
        nc.tensor.matmul(p_Te[:, sl], t_lhemp[:], t_PTs_lo[:, sl], start=True, stop=True)
    mm_emp.out = p_Te
    term_chunked(0, t_b2[:, 0:1], t_bn[:, 0:1], mm_emp)

    p_Ts = psT.tile([128, N], F32, tag="T")

    def mm_src(ch, sl):
        nc.tensor.matmul(p_Ts[:, sl], t_lh2s[:], t_PTs_lo[:, sl], start=True, stop=False)
        nc.tensor.matmul(p_Ts[:, sl], t_lhm2s[:], t_PTs_hi[:, sl], start=False, stop=True)
    mm_src.out = p_Ts
    term_chunked(2, t_b2[:, 1:2], t_bn[:, 1:2], mm_src)

    # ---- target one-hot (transposed), masked probs, term --------------------
    p_yt = psP.tile([64, N], F32, tag="prep")
    for a in range(8):
        nc.tensor.transpose(p_yt[:, a * 128:(a + 1) * 128],
                            t_oh[:, a * 64:(a + 1) * 64], t_ident)
    t_YtT = sb.tile([64, N], F32)
    t_nt = sb.tile([64, 1], F32)
    nc.vector.tensor_scalar(t_YtT[:], p_yt[:], 0.0, None,
                            op0=ALU.add, op1=ALU.add, accum_out=t_nt[:])
    t_lh2t = sb.tile([64, 128], BF16)
    nc.vector.tensor_scalar(t_lh2t[:], t_YtT[:, 0:SH], 2.0, None, op0=ALU.mult)
    t_lhm2t = sb.tile([64, 128], BF16)
    nc.vector.tensor_scalar(t_lhm2t[:], t_YtT[:, 0:SH], -2.0, None, op0=ALU.mult)
    p_rbt = psP.tile([128, N], F32, tag="prep")
    for ch in range(2):
        sl = slice(ch * CHUNK, (ch + 1) * CHUNK)
        nc.tensor.matmul(p_rbt[:, sl], t_lbT, t_R2b_t[:, sl], start=True, stop=True)
    t_RMt = sb.tile([128, N], BF16)
    nc.vector.scalar_tensor_tensor(t_RMt[0:64, :], t_YtT[:], 0.0, p_rbt[0:64, :],
                                   op0=ALU.is_equal, op1=ALU.mult)
    nc.vector.scalar_tensor_tensor(t_RMt[64:128, :], t_YtT[:], 0.0, p_rbt[64:128, :],
                                   op0=ALU.is_equal, op1=ALU.mult)
    t_PTt_lo = sb.tile([64, N], BF16)
    nc.vector.tensor_tensor(t_PTt_lo[:], t_Ext[0:64, :], t_RMt[0:64, :], op=ALU.mult)
    t_PTt_hi = sb.tile([64, N], BF16)
    nc.vector.tensor_tensor(t_PTt_hi[:], t_Ext[64:128, :], t_RMt[64:128, :], op=ALU.mult)

    p_Tt = psT.tile([128, N], F32, tag="T")

    def mm_tgt(ch, sl):
        nc.tensor.matmul(p_Tt[:, sl], t_lh2t[:], t_PTt_lo[:, sl], start=True, stop=False)
        nc.tensor.matmul(p_Tt[:, sl], t_lhm2t[:], t_PTt_hi[:, sl], start=False, stop=True)
    mm_tgt.out = p_Tt
    term_chunked(4, t_b2[:, 2:3], t_bn[:, 2:3], mm_tgt)

    # ---- L0 corrections, fac weights, gathers -------------------------------
    t_w0 = sb.tile([128, 3], F32)
    nc.scalar.activation(t_w0[:], t_q0[:], AF.Exp, bias=t_epsb[:], scale=1.0)
    t_wi0 = sb.tile([128, 3], F32)
    nc.vector.reciprocal(t_wi0[:], t_w0[:])
    t_z0 = sb.tile([128, 3], F32)
    nc.vector.tensor_tensor(t_z0[:], t_w0[:], t_wi0[:], op=ALU.add)
    t_L0 = sb.tile([128, 3], F32)
    nc.scalar.activation(t_L0[:], t_z0[:], AF.Ln, bias=t_c0b[:], scale=SE)

    def fac_of(n_ap):
        t1 = sb.tile([64, 1], F32, tag="fac_t1")
        nc.vector.tensor_scalar(t1[:], n_ap, -1.0, float(N), op0=ALU.mult, op1=ALU.add)
        t2 = sb.tile([64, 1], F32, tag="fac_t2")
        nc.vector.tensor_tensor(t2[:], t1[:], n_ap, op=ALU.mult)   # n*(N-n)
        t3 = sb.tile([64, 1], F32, tag="fac_t3")
        nc.vector.tensor_scalar(t3[:], t2[:], 1.0, None, op0=ALU.max)
        rec = sb.tile([64, 1], F32, tag="fac_rec")
        nc.vector.reciprocal(rec[:], t3[:])
        g = sb.tile([64, 1], F32, tag="fac_g")
        nc.vector.tensor_scalar(g[:], t2[:], 0.5, None, op0=ALU.is_gt)
        fac = sb.tile([64, 1], F32, tag="fac_out")
        nc.vector.tensor_tensor(fac[:], rec[:], g[:], op=ALU.mult)
        return fac

    fac_s = fac_of(t_ns[0:64, :])
    fac_t = fac_of(t_nt[:])
    t_rhs_s = sb.tile([64, 4], F32)
    nc.vector.tensor_scalar(t_rhs_s[:, 0:1], fac_s[:], 0.25, None, op0=ALU.mult)
    nc.vector.tensor_scalar(t_rhs_s[:, 1:2], fac_s[:], -0.5, None, op0=ALU.mult)
    nc.vector.tensor_copy(t_rhs_s[:, 2:3], t_ns[0:64, :])
    nc.vector.tensor_copy(t_rhs_s[:, 3:4], t_ns[0:64, :])
    t_rhs_t = sb.tile([64, 2], F32)
    nc.vector.tensor_scalar(t_rhs_t[:, 0:1], fac_t[:], 0.25 * beta_t, None, op0=ALU.mult)
    nc.vector.tensor_copy(t_rhs_t[:, 1:2], t_nt[:])
    p_gs = psP.tile([128, 4], F32, tag="prep")
    nc.tensor.matmul(p_gs[:], t_YTsx[0:64, 0:SH], t_rhs_s[:], start=True, stop=True)
    t_W3 = sb.tile([128, 3], F32)
    t_N3 = sb.tile([128, 3], F32)
    nc.vector.tensor_copy(t_W3[:, 0:2], p_gs[:, 0:2])
    nc.vector.tensor_copy(t_N3[:, 0:2], p_gs[:, 2:4])
    p_gt = psP.tile([128, 2], F32, tag="prep")
    nc.tensor.matmul(p_gt[:], t_YtT[:, 0:SH], t_rhs_t[:], start=True, stop=True)
    nc.vector.tensor_copy(t_W3[:, 2:3], p_gt[:, 0:1])
    nc.vector.tensor_copy(t_N3[:, 2:3], p_gt[:, 1:2])
    t_corr = sb.tile([128, 3], F32)
    nc.vector.tensor_tensor(t_corr[:], t_L0[:], t_N3[:], op=ALU.mult)

    # ---- final reduction: weighted row sums -> two scalars ------------------
    t_as3 = sb.tile([128, 3], F32)
    nc.vector.tensor_tensor(t_as3[:, 0:1], t_as[:, 0:1], t_as[:, 1:2], op=ALU.add)
    nc.vector.tensor_tensor(t_as3[:, 1:2], t_as[:, 2:3], t_as[:, 3:4], op=ALU.add)
    nc.vector.tensor_tensor(t_as3[:, 2:3], t_as[:, 4:5], t_as[:, 5:6], op=ALU.add)
    t_r3 = sb.tile([128, 3], F32)
    nc.vector.tensor_tensor(t_r3[:], t_as3[:], t_corr[:], op=ALU.subtract)
    t_wr3 = sb.tile([128, 3], F32)
    nc.vector.tensor_tensor(t_wr3[:], t_W3[:], t_r3[:], op=ALU.mult)
    p_fin = psT.tile([1, 3], F32, tag="T")
    nc.tensor.matmul(p_fin[:], t_ones[:], t_wr3[:], start=True, stop=True)
    t_fin = sb.tile([1, 3], F32)
    nc.vector.tensor_copy(t_fin[:], p_fin[:])
    t_o = sb.tile([1, 2], F32)
    nc.vector.tensor_copy(t_o[:, 0:1], t_fin[:, 0:1])
    nc.vector.tensor_tensor(t_o[:, 1:2], t_fin[:, 1:2], t_fin[:, 2:3], op=ALU.add)
    nc.sync.dma_start(out2[:], t_o[:])


_NC_CACHE = {}


def _get_nc(beta_t: float):
    key = beta_t
    if key in _NC_CACHE:
        return _NC_CACHE[key]
    _patch_act_tables()
    nc = bacc.Bacc("TRN2", target_bir_lowering=False, debug=False, num_devices=NCORES)
    ins = {name: nc.dram_tensor(name, shape, dt, kind="ExternalInput").ap()
           for name, (shape, dt) in INPUT_SPECS.items()}
    out2 = nc.dram_tensor("out2", [1, 2], F32, kind="ExternalOutput").ap()
    with tile.TileContext(nc) as tc:
        _body(tc, ins, out2, beta_t)
    nc.compile()
    _NC_CACHE[key] = nc
    return nc


def make_in_maps(y_s, y_s_adv, labels_s, y_t, y_t_adv):
    bf16 = mybir.dt.np(BF16)
    lab = np.asarray(labels_s).astype(np.float32)
    lhA = np.zeros((128, 4), bf16); lhA[0:64, 0] = 1.0; lhA[64:128, 1] = 1.0
    lhB = np.zeros((128, 4), bf16); lhB[0:64, 2] = 1.0; lhB[64:128, 3] = 1.0
    lbS = np.zeros((2, 128), bf16); lbS[0, 0:64] = 1.0; lbS[1, 64:128] = 1.0
    lbT = np.zeros((2, 128), bf16); lbT[0, 0:64] = 1.0; lbT[1, 64:128] = 1.0
    smalls = np.zeros((128, 134), np.float32)
    smalls[:, 0:128] = np.eye(128, dtype=np.float32)
    smalls[:, 129:130] = (np.arange(128) % 64).astype(np.float32).reshape(128, 1)
    smalls[:, 130:132] = lhA.view(np.float32)
    smalls[:, 132:134] = lhB.view(np.float32)
    rows2 = np.concatenate([lbS, lbT], 1).view(np.float32)
    arrs = [np.ascontiguousarray(np.asarray(a, dtype=np.float32))
            for a in (y_s, y_s_adv, y_t, y_t_adv)]
    in_maps = []
    for k in range(NCORES):
        sh = k * SH
        ys_r, ysa_r, yt_r, yta_r = [np.roll(a, -sh, axis=0) for a in arrs]
        lab_r = np.roll(lab, -sh)
        sm = smalls.copy()
        sm[:, 128] = lab_r[:SH]
        rows1 = np.zeros((1, 1152), bf16)
        rows1[0, 0:N] = lab_r
        rows1[0, N:N + 128] = 1.0
        in_maps.append({
            "yT_sx": np.ascontiguousarray(
                np.concatenate([ys_r.T, ysa_r.T], 0).astype(bf16)),
            "yT_tx": np.ascontiguousarray(
                np.concatenate([yt_r.T, yta_r.T], 0).astype(bf16)),
            "yta_nat": np.ascontiguousarray(
                yta_r.reshape(8, 128, 64).transpose(1, 0, 2).reshape(128, 512)),
            "y4_nat": np.ascontiguousarray(
                np.concatenate([ys_r[:SH], ysa_r[:SH], yt_r[:SH], yta_r[:SH]], 1)),
            "smalls": sm, "rows1": rows1, "rows2": rows2,
        })
    return in_maps


def kernel(y_s, y_s_adv, labels_s, y_t, y_t_adv, epoch, _trace=False):
    beta_t = 1.0 if int(np.asarray(epoch)) >= 10 else 0.0
    nc = _get_nc(beta_t)
    in_maps = make_in_maps(y_s, y_s_adv, labels_s, y_t, y_t_adv)
    res = run_bass_kernel_spmd(nc, in_maps, core_ids=list(range(NCORES)),
                               trace=_trace)
    tot = np.zeros(2, dtype=np.float64)
    for r in res.results:
        tot += r["out2"].reshape(2).astype(np.float64)
    out = (np.float32(tot[0]), np.float32(tot[1]))
    if _trace:
        return out, res
    return out


# revision 30
# speedup vs baseline: 1.0045x; 1.0045x over previous
"""Trainium2 Bass kernel for nn_AUCDomainAdapation (AUC domain-adaptation loss).

Contract: kernel(**inputs) takes the FULL unsharded inputs of reference.setup_inputs()
and returns the same structure as reference.reference(**inputs): a tuple
(0.25*empirical, transfer) of float32 scalars.

Math (validated vs reference):
  For sample i with label c_i, the masked pairwise sum per term reduces to
    sum_j M_ij * L(Q_ij),  Q_ij = q0_i + s * B'_ij
  where B'_ij = (s*Y_shard @ pTm)_ij and pTm[c, j] = p[j, c] * [label_j != c]
  (class-major softmax probs with same-label columns zeroed).  Masked entries
  hit B' = 0 exactly, so the unmasked row-sum minus n_{c_i} * L(q0_i) is the
  masked sum.  L(Q) = softplus(-Q) + softplus(Q + 2*eps)
              = ln((1 + e^{2 eps}) + e^{Q+2 eps} + e^{-Q}).

Sharding: row-shard the 1024 samples over 8 cores (128 rows each).  Inputs are
rotated per-core along the sample axis so a single SPMD program (slice [0:128])
serves all cores; per-core scalar partials are summed on the host.
"""

import numpy as np
from contextlib import ExitStack

import concourse.bass as bass
import concourse.bacc as bacc
import concourse.tile as tile
from concourse import mybir
from concourse.bass_utils import run_bass_kernel_spmd
from concourse._compat import with_exitstack

F32 = mybir.dt.float32
BF16 = mybir.dt.bfloat16
AF = mybir.ActivationFunctionType
ALU = mybir.AluOpType

N = 1024          # samples
C = 64            # classes
SH = 128          # rows per core
NCORES = 8
EPS = 0.05
C0 = float(1.0 + np.exp(2 * EPS))   # 1 + e^{2eps}
SE = float(np.exp(EPS))             # e^{eps}
CHUNK = 512                         # matmul moving free-dim max

INPUT_SPECS = {
    "yT_sx":   ([128, N], BF16),   # [y_s^T ; y_s_adv^T] class-major stack
    "yT_tx":   ([128, N], BF16),   # [y_t^T ; y_t_adv^T]
    "yta_nat": ([128, 512], F32),  # y_t_adv natural, j=(a,p): [p, a*64+c]
    "y4_nat":  ([128, 256], F32),  # shard rows: [y_s | y_sa | y_t | y_ta]
    # packed constants:
    # smalls: ident[0:128] | lab_sh[128] | iota_mod[129] | lhA-bits[130:132] | lhB-bits[132:134]
    "smalls":  ([128, 134], F32),
    # rows1: lab_row[0:1024] | ones1[1024:1152] (bf16: labels <= 63 exact)
    "rows1":   ([1, 1152], BF16),
    # rows2: bf16 bits of [lbS | lbT] ([2, 256] bf16)
    "rows2":   ([2, 128], F32),
}


def _patch_act_tables():
    """Force exp+ln to resolve to the single natural_log_exp_and_others set,
    avoiding table reloads between Exp and Ln activations."""
    if getattr(bacc, "_act_tables_patched", False):
        return
    orig = bacc.get_activation_tables

    def patched(arch):
        tabs = dict(orig(arch))
        out = {}
        for name, funcs in tabs.items():
            if name != "natural_log_exp_and_others":
                funcs = {f for f in funcs if f not in (AF.Exp, AF.Ln)}
            out[name] = funcs
        return out

    bacc.get_activation_tables = patched
    bacc._act_tables_patched = True


@with_exitstack
def _body(ctx: ExitStack, tc: "tile.TileContext", ins: dict, out2: "bass.AP", beta_t: float):
    nc = tc.nc
    sb = ctx.enter_context(tc.tile_pool(name="sb", bufs=1))
    sw = ctx.enter_context(tc.tile_pool(name="sw", bufs=3))   # loss scratch
    psT = ctx.enter_context(tc.tile_pool(name="psT", bufs=2, space="PSUM"))
    psP = ctx.enter_context(tc.tile_pool(name="psP", bufs=2, space="PSUM"))

    # ---- input loads (packed; spread over both HWDGE engines) --------------
    t_ysx = sb.tile([128, N], BF16)
    nc.sync.dma_start(t_ysx[:], ins["yT_sx"][:])
    t_ytx = sb.tile([128, N], BF16)
    nc.sync.dma_start(t_ytx[:], ins["yT_tx"][:])
    t_rows1 = sb.tile([1, 1152], BF16)
    nc.gpsimd.dma_start(t_rows1[:], ins["rows1"][:])
    t_smalls = sb.tile([128, 134], F32)
    nc.gpsimd.dma_start(t_smalls[:], ins["smalls"][:])
    t_ytan = sb.tile([128, 512], F32)
    nc.gpsimd.dma_start(t_ytan[:], ins["yta_nat"][:])
    t_y4 = sb.tile([128, 256], F32)
    nc.gpsimd.dma_start(t_y4[:], ins["y4_nat"][:])
    t_rows2 = sb.tile([2, 128], F32)
    nc.gpsimd.dma_start(t_rows2[:], ins["rows2"][:])
    t_ident = t_smalls[:, 0:128]
    t_labsh = t_smalls[:, 128:129]
    t_iotam = t_smalls[:, 129:130]
    t_lhA = t_smalls[:, 130:132].bitcast(BF16)
    t_lhB = t_smalls[:, 132:134].bitcast(BF16)
    t_lab1 = t_rows1[0:1, 0:N]
    t_ones1 = t_rows1[0:1, N:N + 128]
    _rows2b = t_rows2[:].bitcast(BF16)
    t_lbS = _rows2b[:, 0:128]
    t_lbT = _rows2b[:, 128:256]

    # ---- small constants --------------------------------------------------
    t_epsb = sb.tile([128, 1], F32)
    nc.gpsimd.memset(t_epsb[:], EPS)
    t_c0b = sb.tile([128, 1], F32)
    nc.gpsimd.memset(t_c0b[:], C0)
    t_ones = sb.tile([128, 1], F32)
    nc.gpsimd.memset(t_ones[:], 1.0)

    # ---- source exp + colsum + reciprocal chain (critical path) -------------
    t_Exs = sb.tile([128, N], BF16)
    nc.scalar.activation(t_Exs[:], t_ysx[:], AF.Exp)
    t_ey4 = sb.tile([128, 256], F32)
    nc.scalar.activation(t_ey4[:], t_y4[:], AF.Exp)
    p_cs_s = psP.tile([2, N], F32, tag="prep")
    for ch in range(2):
        sl = slice(ch * CHUNK, (ch + 1) * CHUNK)
        nc.tensor.matmul(p_cs_s[:, sl], t_lhA[:, 0:2], t_Exs[:, sl], start=True, stop=True)
    t_lnS_s = sb.tile([2, N], F32)
    t_R2b_s = sb.tile([2, N], BF16)
    for ch in range(2):
        sl = slice(ch * CHUNK, (ch + 1) * CHUNK)
        nc.scalar.activation(t_lnS_s[:, sl], p_cs_s[:, sl], AF.Ln)
        nc.scalar.activation(t_R2b_s[:, sl], t_lnS_s[:, sl], AF.Exp, scale=-1.0)

    # ---- per-row chain: a_i, q0, exp biases (gates the first loss exp) ------
    t_m8 = sb.tile([128, 8], F32)
    ytan_v = t_ytan[:].rearrange("p (a c) -> p a c", c=64)
    nc.vector.tensor_reduce(t_m8[:], ytan_v, axis=mybir.AxisListType.X, op=ALU.max)
    t_oh = sb.tile([128, 512], F32)
    nc.vector.tensor_tensor(t_oh[:].rearrange("p (a c) -> p a c", c=64), ytan_v,
                            t_m8[:, :, None].broadcast_to((128, 8, 64)),
                            op=ALU.is_equal)
    t_den4 = sb.tile([128, 4], F32)
    nc.vector.tensor_reduce(t_den4[:], t_ey4[:].rearrange("p (a c) -> p a c", c=64),
                            axis=mybir.AxisListType.X, op=ALU.add)
    t_rec4 = sb.tile([128, 4], F32)
    nc.vector.reciprocal(t_rec4[:], t_den4[:])
    t_iotaf = sb.tile([128, 64], F32)
    nc.gpsimd.iota(t_iotaf[:], pattern=[[1, 64]], base=0, channel_multiplier=0,
                   allow_small_or_imprecise_dtypes=True)
    t_Ysn = sb.tile([128, 64], F32)
    nc.vector.tensor_scalar(t_Ysn[:], t_iotaf[:], t_labsh, None, op0=ALU.is_equal)
    t_Y4n = sb.tile([128, 256], F32)
    nc.gpsimd.tensor_copy(t_Y4n[:, 0:64], t_Ysn[:])
    nc.gpsimd.tensor_copy(t_Y4n[:, 64:128], t_Ysn[:])
    nc.gpsimd.tensor_copy(t_Y4n[:, 128:192], t_oh[:, 0:64])
    nc.gpsimd.tensor_copy(t_Y4n[:, 192:256], t_oh[:, 0:64])
    t_scr = sb.tile([128, 256], F32)
    nc.gpsimd.tensor_tensor(t_scr[:], t_Y4n[:], t_y4[:], op=ALU.mult)
    t_sel4 = sb.tile([128, 4], F32)
    nc.vector.tensor_reduce(t_sel4[:], t_scr[:].rearrange("p (a c) -> p a c", c=64),
                            axis=mybir.AxisListType.X, op=ALU.add)
    t_es4 = sb.tile([128, 4], F32)
    nc.scalar.activation(t_es4[:], t_sel4[:], AF.Exp)
    t_a4 = sb.tile([128, 4], F32)
    nc.vector.tensor_tensor(t_a4[:], t_es4[:], t_rec4[:], op=ALU.mult)
    t_q0 = sb.tile([128, 3], F32)
    nc.vector.tensor_scalar(t_q0[:, 0:1], t_a4[:, 0:1], -4.0, 4.0 - EPS,
                            op0=ALU.mult, op1=ALU.add)
    t_u = sb.tile([128, 1], F32, tag="u1")
    nc.vector.tensor_tensor(t_u[:], t_a4[:, 1:2], t_a4[:, 0:1], op=ALU.subtract)
    nc.vector.tensor_scalar(t_q0[:, 1:2], t_u[:], 2.0, -EPS, op0=ALU.mult, op1=ALU.add)
    t_u2 = sb.tile([128, 1], F32, tag="u2")
    nc.vector.tensor_tensor(t_u2[:], t_a4[:, 3:4], t_a4[:, 2:3], op=ALU.subtract)
    nc.vector.tensor_scalar(t_q0[:, 2:3], t_u2[:], 2.0, -EPS, op0=ALU.mult, op1=ALU.add)
    t_b1e = sb.tile([128, 1], F32)
    nc.vector.tensor_scalar(t_b1e[:], t_q0[:, 0:1], EPS, None, op0=ALU.add)
    t_b2 = sb.tile([128, 3], F32)
    nc.vector.tensor_scalar(t_b2[:], t_q0[:], 2.0 * EPS, None, op0=ALU.add)
    t_bn = sb.tile([128, 3], F32)
    nc.vector.tensor_scalar(t_bn[:], t_q0[:], -1.0, None, op0=ALU.mult)

    # ---- target exp + colsum + reciprocal chain -----------------------------
    t_Ext = sb.tile([128, N], BF16)
    nc.scalar.activation(t_Ext[:], t_ytx[:], AF.Exp)
    p_cs_t = psP.tile([2, N], F32, tag="prep")
    for ch in range(2):
        sl = slice(ch * CHUNK, (ch + 1) * CHUNK)
        nc.tensor.matmul(p_cs_t[:, sl], t_lhB[:, 2:4], t_Ext[:, sl], start=True, stop=True)
    t_lnS_t = sb.tile([2, N], F32)
    nc.scalar.activation(t_lnS_t[:], p_cs_t[:], AF.Ln)
    t_R2b_t = sb.tile([2, N], BF16)
    nc.scalar.activation(t_R2b_t[:], t_lnS_t[:], AF.Exp, scale=-1.0)

    # ---- source one-hot stack (label broadcast via PE, borrowed T slot) -----
    p_lab128 = psT.tile([128, N], F32, tag="T")
    for ch in range(2):
        sl = slice(ch * CHUNK, (ch + 1) * CHUNK)
        nc.tensor.matmul(p_lab128[:, sl], t_ones1, t_lab1[:, sl], start=True, stop=True)
    t_YTsx = sb.tile([128, N], F32)
    t_ns = sb.tile([128, 1], F32)
    nc.vector.tensor_scalar(t_YTsx[:], p_lab128[:], t_iotam, None,
                            op0=ALU.is_equal, op1=ALU.add, accum_out=t_ns[:])
    t_lhemp = sb.tile([64, 128], BF16)
    nc.vector.tensor_scalar(t_lhemp[:], t_YTsx[0:64, 0:SH], 4.0, None, op0=ALU.mult)
    t_lh2s = sb.tile([64, 128], BF16)
    nc.vector.tensor_scalar(t_lh2s[:], t_YTsx[0:64, 0:SH], 2.0, None, op0=ALU.mult)
    t_lhm2s = sb.tile([64, 128], BF16)
    nc.vector.tensor_scalar(t_lhm2s[:], t_YTsx[0:64, 0:SH], -2.0, None, op0=ALU.mult)

    # ---- source masked probs -------------------------------------------------
    p_rbs = psP.tile([128, N], F32, tag="prep")
    for ch in range(2):
        sl = slice(ch * CHUNK, (ch + 1) * CHUNK)
        nc.tensor.matmul(p_rbs[:, sl], t_lbS, t_R2b_s[:, sl], start=True, stop=True)
    t_RMs = sb.tile([128, N], BF16)
    t_PTs_lo = sb.tile([64, N], BF16)
    t_PTs_hi = sb.tile([64, N], BF16)
    for ch in range(2):
        sl = slice(ch * CHUNK, (ch + 1) * CHUNK)
        nc.vector.scalar_tensor_tensor(t_RMs[:, sl], t_YTsx[:, sl], 0.0, p_rbs[:, sl],
                                       op0=ALU.is_equal, op1=ALU.mult)
        nc.vector.tensor_tensor(t_PTs_lo[:, sl], t_Exs[0:64, sl], t_RMs[0:64, sl], op=ALU.mult)
        nc.vector.tensor_tensor(t_PTs_hi[:, sl], t_Exs[64:128, sl], t_RMs[64:128, sl], op=ALU.mult)

    # ---- emp + src terms (start while target prep continues) ----------------
    t_as = sb.tile([128, 6], F32)

    def term_chunked(base_col, bias2_ap, biasn_ap, mm_emit):
        t_eu = sw.tile([128, N], F32, tag="eu")
        t_ev = sw.tile([128, N], F32, tag="ev")
        t_z = sw.tile([128, N], F32, tag="z")
        t_l = sw.tile([128, N], F32, tag="l")
        for ch in range(2):
            sl = slice(ch * CHUNK, (ch + 1) * CHUNK)
            mm_emit(ch, sl)
            nc.scalar.activation(t_eu[:, sl], mm_emit.out[:, sl], AF.Exp,
                                 bias=bias2_ap, scale=1.0)
            nc.scalar.activation(t_ev[:, sl], mm_emit.out[:, sl], AF.Exp,
                                 bias=biasn_ap, scale=-1.0)
            nc.vector.tensor_tensor(t_z[:, sl], t_eu[:, sl], t_ev[:, sl], op=ALU.add)
            nc.scalar.activation(t_l[:, sl], t_z[:, sl], AF.Ln, bias=t_c0b[:], scale=1.0,
                                 accum_out=t_as[:, base_col + ch:base_col + ch + 1])

    p_Te = psT.tile([128, N], F32, tag="T")

    def mm_emp(ch, sl):
        nc.tensor.matmul(p_Te[:, sl], t_lhemp[:], t_PTs_lo[:, sl], start=True, stop=True)
    mm_emp.out = p_Te
    term_chunked(0, t_b2[:, 0:1], t_bn[:, 0:1], mm_emp)

    p_Ts = psT.tile([128, N], F32, tag="T")

    def mm_src(ch, sl):
        nc.tensor.matmul(p_Ts[:, sl], t_lh2s[:], t_PTs_lo[:, sl], start=True, stop=False)
        nc.tensor.matmul(p_Ts[:, sl], t_lhm2s[:], t_PTs_hi[:, sl], start=False, stop=True)
    mm_src.out = p_Ts
    term_chunked(2, t_b2[:, 1:2], t_bn[:, 1:2], mm_src)

    # ---- target one-hot (transposed), masked probs, term --------------------
    p_yt = psP.tile([64, N], F32, tag="prep")
    for a in range(8):
        nc.tensor.transpose(p_yt[:, a * 128:(a + 1) * 128],
                            t_oh[:, a * 64:(a + 1) * 64], t_ident)
    t_YtT = sb.tile([64, N], F32)
    t_nt = sb.tile([64, 1], F32)
    nc.vector.tensor_scalar(t_YtT[:], p_yt[:], 0.0, None,
                            op0=ALU.add, op1=ALU.add, accum_out=t_nt[:])
    t_lh2t = sb.tile([64, 128], BF16)
    nc.vector.tensor_scalar(t_lh2t[:], t_YtT[:, 0:SH], 2.0, None, op0=ALU.mult)
    t_lhm2t = sb.tile([64, 128], BF16)
    nc.vector.tensor_scalar(t_lhm2t[:], t_YtT[:, 0:SH], -2.0, None, op0=ALU.mult)
    p_rbt = psP.tile([128, N], F32, tag="prep")
    for ch in range(2):
        sl = slice(ch * CHUNK, (ch + 1) * CHUNK)
        nc.tensor.matmul(p_rbt[:, sl], t_lbT, t_R2b_t[:, sl], start=True, stop=True)
    t_RMt = sb.tile([128, N], BF16)
    nc.vector.scalar_tensor_tensor(t_RMt[0:64, :], t_YtT[:], 0.0, p_rbt[0:64, :],
                                   op0=ALU.is_equal, op1=ALU.mult)
    nc.vector.scalar_tensor_tensor(t_RMt[64:128, :], t_YtT[:], 0.0, p_rbt[64:128, :],
                                   op0=ALU.is_equal, op1=ALU.mult)
    t_PTt_lo = sb.tile([64, N], BF16)
    nc.vector.tensor_tensor(t_PTt_lo[:], t_Ext[0:64, :], t_RMt[0:64, :], op=ALU.mult)
    t_PTt_hi = sb.tile([64, N], BF16)
    nc.vector.tensor_tensor(t_PTt_hi[:], t_Ext[64:128, :], t_RMt[64:128, :], op=ALU.mult)

    p_Tt = psT.tile([128, N], F32, tag="T")

    def mm_tgt(ch, sl):
        nc.tensor.matmul(p_Tt[:, sl], t_lh2t[:], t_PTt_lo[:, sl], start=True, stop=False)
        nc.tensor.matmul(p_Tt[:, sl], t_lhm2t[:], t_PTt_hi[:, sl], start=False, stop=True)
    mm_tgt.out = p_Tt
    term_chunked(4, t_b2[:, 2:3], t_bn[:, 2:3], mm_tgt)

    # ---- L0 corrections, fac weights, gathers -------------------------------
    t_w0 = sb.tile([128, 3], F32)
    nc.scalar.activation(t_w0[:], t_q0[:], AF.Exp, bias=t_epsb[:], scale=1.0)
    t_wi0 = sb.tile([128, 3], F32)
    nc.vector.reciprocal(t_wi0[:], t_w0[:])
    t_z0 = sb.tile([128, 3], F32)
    nc.vector.tensor_tensor(t_z0[:], t_w0[:], t_wi0[:], op=ALU.add)
    t_L0 = sb.tile([128, 3], F32)
    nc.scalar.activation(t_L0[:], t_z0[:], AF.Ln, bias=t_c0b[:], scale=SE)

    def fac_of(n_ap):
        t1 = sb.tile([64, 1], F32, tag="fac_t1")
        nc.vector.tensor_scalar(t1[:], n_ap, -1.0, float(N), op0=ALU.mult, op1=ALU.add)
        t2 = sb.tile([64, 1], F32, tag="fac_t2")
        nc.vector.tensor_tensor(t2[:], t1[:], n_ap, op=ALU.mult)   # n*(N-n)
        t3 = sb.tile([64, 1], F32, tag="fac_t3")
        nc.vector.tensor_scalar(t3[:], t2[:], 1.0, None, op0=ALU.max)
        rec = sb.tile([64, 1], F32, tag="fac_rec")
        nc.vector.reciprocal(rec[:], t3[:])
        g = sb.tile([64, 1], F32, tag="fac_g")
        nc.vector.tensor_scalar(g[:], t2[:], 0.5, None, op0=ALU.is_gt)
        fac = sb.tile([64, 1], F32, tag="fac_out")
        nc.vector.tensor_tensor(fac[:], rec[:], g[:], op=ALU.mult)
        return fac

    fac_s = fac_of(t_ns[0:64, :])
    fac_t = fac_of(t_nt[:])
    t_rhs_s = sb.tile([64, 4], F32)
    nc.vector.tensor_scalar(t_rhs_s[:, 0:1], fac_s[:], 0.25, None, op0=ALU.mult)
    nc.vector.tensor_scalar(t_rhs_s[:, 1:2], fac_s[:], -0.5, None, op0=ALU.mult)
    nc.vector.tensor_copy(t_rhs_s[:, 2:3], t_ns[0:64, :])
    nc.vector.tensor_copy(t_rhs_s[:, 3:4], t_ns[0:64, :])
    t_rhs_t = sb.tile([64, 2], F32)
    nc.vector.tensor_scalar(t_rhs_t[:, 0:1], fac_t[:], 0.25 * beta_t, None, op0=ALU.mult)
    nc.vector.tensor_copy(t_rhs_t[:, 1:2], t_nt[:])
    p_gs = psP.tile([128, 4], F32, tag="prep")
    nc.tensor.matmul(p_gs[:], t_YTsx[0:64, 0:SH], t_rhs_s[:], start=True, stop=True)
    t_W3 = sb.tile([128, 3], F32)
    t_N3 = sb.tile([128, 3], F32)
    nc.vector.tensor_copy(t_W3[:, 0:2], p_gs[:, 0:2])
    nc.vector.tensor_copy(t_N3[:, 0:2], p_gs[:, 2:4])
    p_gt = psP.tile([128, 2], F32, tag="prep")
    nc.tensor.matmul(p_gt[:], t_YtT[:, 0:SH], t_rhs_t[:], start=True, stop=True)
    nc.vector.tensor_copy(t_W3[:, 2:3], p_gt[:, 0:1])
    nc.vector.tensor_copy(t_N3[:, 2:3], p_gt[:, 1:2])
    t_corr = sb.tile([128, 3], F32)
    nc.vector.tensor_tensor(t_corr[:], t_L0[:], t_N3[:], op=ALU.mult)

    # ---- final reduction: weighted row sums -> two scalars ------------------
    t_as3 = sb.tile([128, 3], F32)
    nc.vector.tensor_tensor(t_as3[:, 0:1], t_as[:, 0:1], t_as[:, 1:2], op=ALU.add)
    nc.vector.tensor_tensor(t_as3[:, 1:2], t_as[:, 2:3], t_as[:, 3:4], op=ALU.add)
    nc.vector.tensor_tensor(t_as3[:, 2:3], t_as[:, 4:5], t_as[:, 5:6], op=ALU.add)
    t_r3 = sb.tile([128, 3], F32)
    nc.vector.tensor_tensor(t_r3[:], t_as3[:], t_corr[:], op=ALU.subtract)
    t_wr3 = sb.tile([128, 3], F32)
    nc.vector.tensor_tensor(t_wr3[:], t_W3[:], t_r3[:], op=ALU.mult)
    p_fin = psT.tile([1, 3], F32, tag="T")
    nc.tensor.matmul(p_fin[:], t_ones[:], t_wr3[:], start=True, stop=True)
    t_fin = sb.tile([1, 3], F32)
    nc.vector.tensor_copy(t_fin[:], p_fin[:])
    t_o = sb.tile([1, 2], F32)
    nc.vector.tensor_copy(t_o[:, 0:1], t_fin[:, 0:1])
    nc.vector.tensor_tensor(t_o[:, 1:2], t_fin[:, 1:2], t_fin[:, 2:3], op=ALU.add)
    nc.sync.dma_start(out2[:], t_o[:])


_NC_CACHE = {}


def _get_nc(beta_t: float):
    key = beta_t
    if key in _NC_CACHE:
        return _NC_CACHE[key]
    _patch_act_tables()
    nc = bacc.Bacc("TRN2", target_bir_lowering=False, debug=False, num_devices=NCORES)
    ins = {name: nc.dram_tensor(name, shape, dt, kind="ExternalInput").ap()
           for name, (shape, dt) in INPUT_SPECS.items()}
    out2 = nc.dram_tensor("out2", [1, 2], F32, kind="ExternalOutput").ap()
    with tile.TileContext(nc) as tc:
        _body(tc, ins, out2, beta_t)
    nc.compile()
    _NC_CACHE[key] = nc
    return nc


def make_in_maps(y_s, y_s_adv, labels_s, y_t, y_t_adv):
    bf16 = mybir.dt.np(BF16)
    lab = np.asarray(labels_s).astype(np.float32)
    lhA = np.zeros((128, 4), bf16); lhA[0:64, 0] = 1.0; lhA[64:128, 1] = 1.0
    lhB = np.zeros((128, 4), bf16); lhB[0:64, 2] = 1.0; lhB[64:128, 3] = 1.0
    lbS = np.zeros((2, 128), bf16); lbS[0, 0:64] = 1.0; lbS[1, 64:128] = 1.0
    lbT = np.zeros((2, 128), bf16); lbT[0, 0:64] = 1.0; lbT[1, 64:128] = 1.0
    smalls = np.zeros((128, 134), np.float32)
    smalls[:, 0:128] = np.eye(128, dtype=np.float32)
    smalls[:, 129:130] = (np.arange(128) % 64).astype(np.float32).reshape(128, 1)
    smalls[:, 130:132] = lhA.view(np.float32)
    smalls[:, 132:134] = lhB.view(np.float32)
    rows2 = np.concatenate([lbS, lbT], 1).view(np.float32)
    arrs = [np.ascontiguousarray(np.asarray(a, dtype=np.float32))
            for a in (y_s, y_s_adv, y_t, y_t_adv)]
    in_maps = []
    for k in range(NCORES):
        sh = k * SH
        ys_r, ysa_r, yt_r, yta_r = [np.roll(a, -sh, axis=0) for a in arrs]
        lab_r = np.roll(lab, -sh)
        sm = smalls.copy()
        sm[:, 128] = lab_r[:SH]
        rows1 = np.zeros((1, 1152), bf16)
        rows1[0, 0:N] = lab_r
        rows1[0, N:N + 128] = 1.0
        in_maps.append({
            "yT_sx": np.ascontiguousarray(
                np.concatenate([ys_r.T, ysa_r.T], 0).astype(bf16)),
            "yT_tx": np.ascontiguousarray(
                np.concatenate([yt_r.T, yta_r.T], 0).astype(bf16)),
            "yta_nat": np.ascontiguousarray(
                yta_r.reshape(8, 128, 64).transpose(1, 0, 2).reshape(128, 512)),
            "y4_nat": np.ascontiguousarray(
                np.concatenate([ys_r[:SH], ysa_r[:SH], yt_r[:SH], yta_r[:SH]], 1)),
            "smalls": sm, "rows1": rows1, "rows2": rows2,
        })
    return in_maps


def kernel(y_s, y_s_adv, labels_s, y_t, y_t_adv, epoch, _trace=False):
    beta_t = 1.0 if int(np.asarray(epoch)) >= 10 else 0.0
    nc = _get_nc(beta_t)
    in_maps = make_in_maps(y_s, y_s_adv, labels_s, y_t, y_t_adv)
    res = run_bass_kernel_spmd(nc, in_maps, core_ids=list(range(NCORES)),
                               trace=_trace)
    tot = np.zeros(2, dtype=np.float64)
    for r in res.results:
        tot += r["out2"].reshape(2).astype(np.float64)
    out = (np.float32(tot[0]), np.float32(tot[1]))
    if _trace:
        return out, res
    return out
